# revision 1
# baseline (speedup 1.0000x reference)
"""AURC loss kernel for Trainium2, sharded across 8 NeuronCores.

Algorithm (matches the reference):
  logp = log_softmax(x);  score = exp(max logp);  loss = -logp[target]
  sort by score desc;  result = sum_i cumsum(sorted_loss)[i-1]/i / B
Rewritten rank-wise: result = sum_j loss_j * w[rank_j] where
  rank_j = #{m : key_m > key_j},  key = max(x) - logsumexp(x)  (monotone in score)
  w[r] = (H_{B-1} - H_r) / B,  H_r = sum_{i=1}^r 1/i   (precomputed table)

Sharding: batch B=8192 split 1024 rows/core. Each core streams its
[1024, 32000] shard once (row max via DVE, exp-sum via the ACT accumulator,
target logit via indirect DMA). The 8192 key scalars are AllGathered in two
halves: the first half mid-stream so its transfer (PE ones-matmul
replication) and the rank-compares against it hide under the stream, the
second right at the end with compares overlapping the collective's flight.
Each core counts ranks for its own 1024 keys (DVE is_gt+accum and ACT
sign+accum split), evaluates w(rank) analytically from the asymptotic
harmonic expansion, and emits a partial dot product with its local losses.
Host sums the 8 partials.
"""
import sys

if "/opt/trn_rl_repo" not in sys.path:
    sys.path.insert(0, "/opt/trn_rl_repo")

import numpy as np

B, C = 8192, 32000
NCORES = 8
BL = B // NCORES          # rows per core
P = 128                   # partitions
NG = BL // P              # row-groups per core
HG = NG // 2              # groups per AllGather half
CF = 8000                 # columns per streamed chunk
NCH = C // CF             # chunks per row-group
HB = B // 2               # keys per gathered half

_CACHE = {}


def _w_table() -> np.ndarray:
    h = np.zeros(B, dtype=np.float64)
    h[1:] = np.cumsum(1.0 / np.arange(1, B, dtype=np.float64))
    return ((h[B - 1] - h) / B).astype(np.float32)


def _build(debug: bool = False):
    import concourse.bass as bass
    import concourse.bacc as bacc
    import concourse.mybir as mybir
    import concourse.tile as tile

    nc = bacc.Bacc(num_devices=NCORES)
    x = nc.dram_tensor("x", [BL, C], mybir.dt.float32, kind="ExternalInput")
    # flat element offsets r*C + tgt[r], laid out so tile[p, g] = row g*P+p
    toff = nc.dram_tensor("toff", [BL], mybir.dt.int32, kind="ExternalInput")
    out = nc.dram_tensor("out", [1, 1], mybir.dt.float32, kind="ExternalOutput")

    xflat = x.rearrange("a b -> (a b)").unsqueeze(1)
    f32 = mybir.dt.float32
    bf16 = mybir.dt.bfloat16
    i32 = mybir.dt.int32
    AX = mybir.AxisListType.X
    OP = mybir.AluOpType
    AF = mybir.ActivationFunctionType

    with tile.TileContext(nc) as tc:
        with (
            tc.tile_pool(name="sb", bufs=4) as sb,
            tc.tile_pool(name="sm", bufs=1) as sm,
            tc.tile_pool(name="ps", bufs=1, space="PSUM") as ps,
            tc.tile_pool(name="dr", bufs=1, space="DRAM") as dr,
        ):
            keys = sm.tile([P, NG], f32)
            losses = sm.tile([P, NG], f32)
            ones_col = sm.tile([P, 1], f32)
            nc.vector.memset(ones_col[:, :], 1.0)

            # gather target logits x[r, tgt[r]] (one offset per partition
            # per indirect DMA)
            off_t = sm.tile([P, NG], i32)
            nc.sync.dma_start(off_t[:, :], toff.rearrange("(p g) -> p g", g=NG))
            xt = sm.tile([P, NG], f32)
            for g in range(NG):
                nc.gpsimd.indirect_dma_start(
                    out=xt[:, g:g + 1], out_offset=None, in_=xflat,
                    in_offset=bass.IndirectOffsetOnAxis(ap=off_t[:, g:g + 1],
                                                        axis=0))

            m_all = sm.tile([P, NG], f32)
            s_all = sm.tile([P, NG], f32)
            lse_all = sm.tile([P, NG], f32)
            negk = sm.tile([P, NG], f32)
            ones_row = sm.tile([1, P], f32)
            nc.vector.memset(ones_row[:, :], 1.0)
            kl_in = [dr.tile([HG * P], f32, name=f"kl_in{h}") for h in range(2)]
            kl_all = [dr.tile([HB], f32, name=f"kl_all{h}") for h in range(2)]
            kl_row = [sm.tile([1, HB], f32, name=f"kl_row{h}") for h in range(2)]
            # replicated halves of the global keys
            greph = [sm.tile([P, HB], f32, name=f"greph{h}") for h in range(2)]
            # per-(group, piece) partial counts: piece 0 = h1, 1..2 = h2a/h2b
            rparts = sm.tile([P, NG, 3], f32)

            def replicate(h, q, on_act):
                # broadcast kl_row[h] cols [q*2048, (q+1)*2048) to all
                # partitions of greph[h] via ones-matmul (keeps HBM free)
                for j in range(q * 2048, (q + 1) * 2048, 512):
                    pt = ps.tile([P, 512], f32, tag="pt", bufs=2,
                                 name=f"pt{h}_{j}")
                    nc.tensor.matmul(pt[:, :], lhsT=ones_row[:, :],
                                     rhs=kl_row[h][:, j:j + 512],
                                     start=True, stop=True)
                    if on_act:
                        nc.scalar.copy(greph[h][:, j:j + 512], pt[:, :])
                    else:
                        nc.vector.tensor_copy(greph[h][:, j:j + 512], pt[:, :])

            def stream_group(g):
                last = g == NG - 1
                bounds = [c * CF for c in range(NCH)] + [C]
                if last:  # small final chunk shortens the closing epilogue
                    bounds = bounds[:-1] + [C - 800, C]
                nck = len(bounds) - 1
                mx = sm.tile([P, nck], f32, tag="mxL" if last else "mx",
                             bufs=3, name=f"mx{g}")
                sms = sm.tile([P, nck], f32, tag="smsL" if last else "sms",
                              bufs=3, name=f"sms{g}")
                for c in range(nck):
                    lo, hi = bounds[c], bounds[c + 1]
                    t = sb.tile([P, CF], f32, tag="t", name=f"t{g}_{c}")
                    nc.sync.dma_start(
                        t[:, :hi - lo], x[g * P:(g + 1) * P, lo:hi])
                    nc.vector.reduce_max(mx[:, c:c + 1], t[:, :hi - lo],
                                         axis=AX)
                    nc.scalar.activation(out=t[:, :hi - lo], in_=t[:, :hi - lo],
                                         func=AF.Exp,
                                         accum_out=sms[:, c:c + 1])
                nc.vector.reduce_max(m_all[:, g:g + 1], mx[:, :], axis=AX)
                nc.vector.reduce_sum(s_all[:, g:g + 1], sms[:, :], axis=AX)

            def half_epilogue(h):
                # lse = ln(S); key = m - lse; AllGather this half's keys
                lo, hi = h * HG, (h + 1) * HG
                nc.scalar.activation(out=lse_all[:, lo:hi], in_=s_all[:, lo:hi],
                                     func=AF.Ln)
                nc.vector.tensor_tensor(out=keys[:, lo:hi],
                                        in0=m_all[:, lo:hi],
                                        in1=lse_all[:, lo:hi], op=OP.subtract)
                nc.vector.tensor_scalar_mul(negk[:, lo:hi], keys[:, lo:hi],
                                            -1.0)
                nc.sync.dma_start(
                    kl_in[h][:].rearrange("(p g) -> p g", g=HG), keys[:, lo:hi])
                nc.gpsimd.collective_compute(
                    "AllGather", OP.bypass,
                    replica_groups=[list(range(NCORES))],
                    ins=[kl_in[h].opt()], outs=[kl_all[h].opt()])

            def compare_unit(g, piece, src, on_act):
                # accumulate count of this group's keys vs one replicated
                # piece of the global keys into rparts[:, g, piece]
                w = src.shape[1]
                acc = rparts[:, g, piece:piece + 1]
                if on_act:
                    t2 = sb.tile([P, w], bf16, tag="t", name=f"ta{g}_{piece}")
                    nc.scalar.activation(
                        out=t2[:, :], in_=src[:, :], func=AF.Sign,
                        bias=negk[:, g:g + 1], accum_out=acc)
                else:
                    t1 = sb.tile([P, w], bf16, tag="t", name=f"td{g}_{piece}")
                    nc.vector.tensor_scalar(
                        out=t1[:, :], in0=src[:, :], scalar1=keys[:, g:g + 1],
                        scalar2=None, op0=OP.is_gt, op1=OP.add, accum_out=acc)

            # ---- stream first half; AllGather its keys mid-stream ----
            for g in range(HG):
                stream_group(g)
            half_epilogue(0)
            stream_group(HG)
            # replicate half-1 keys to all partitions (overlaps the stream;
            # placed late enough that DVE never idles on the AllGather)
            nc.gpsimd.dma_start(kl_row[0][:, :], kl_all[0][:].unsqueeze(0))
            stream_group(HG + 1)
            replicate(0, 0, on_act=False)
            stream_group(HG + 2)
            replicate(0, 1, on_act=False)
            # compares of early groups vs half 1, tucked into stream slack
            for g in range(HG):
                compare_unit(g, 0, greph[0][:, :], on_act=False)
            stream_group(HG + 3)

            half_epilogue(1)
            nc.vector.tensor_tensor(out=losses[:, :], in0=lse_all[:, :],
                                    in1=xt[:, :], op=OP.subtract)

            # during AG2 flight: late groups vs half 1 (all on ACT so every
            # piece of a group uses the same counting semantics)
            for g in range(HG, NG):
                compare_unit(g, 0, greph[0][:, :], on_act=True)

            # replicate half-2 keys; compare each 2048-wide piece as it lands
            nc.gpsimd.dma_start(kl_row[1][:, :], kl_all[1][:].unsqueeze(0))

            ranks = sm.tile([P, NG], f32)
            for q in range(2):
                replicate(1, q, on_act=True)
                for g in range(NG):
                    compare_unit(g, 1 + q,
                                 greph[1][:, q * 2048:(q + 1) * 2048],
                                 on_act=(g >= HG))
                    if q == 1:
                        nc.vector.reduce_sum(ranks[:, g:g + 1],
                                             rparts[:, g, :], axis=AX)
                        if g >= HG:  # sign-counted: rank = (cnt + B-1)/2
                            nc.vector.tensor_scalar(
                                out=ranks[:, g:g + 1], in0=ranks[:, g:g + 1],
                                scalar1=0.5, scalar2=float((B - 1) / 2),
                                op0=OP.mult, op1=OP.add)

            # w(rank) analytically: H_r = ln(r+1) + g - u(0.5 + u/12),
            # u = 1/(r+1);  w = (H_{B-1}-g)/B - (ln(r+1) - u(0.5+u/12))/B
            EUL = 0.5772156649015329
            h_top = float(np.sum(1.0 / np.arange(1, B, dtype=np.float64)))
            C0 = float((h_top - EUL) / B)
            tt = sm.tile([P, NG], f32)
            nc.vector.tensor_scalar_add(tt[:, :], ranks[:, :], 1.0)
            lnt = sm.tile([P, NG], f32)
            nc.scalar.activation(out=lnt[:, :], in_=tt[:, :], func=AF.Ln)
            u = sm.tile([P, NG], f32)
            nc.vector.reciprocal(u[:, :], tt[:, :])
            v = sm.tile([P, NG], f32)
            nc.vector.tensor_scalar(out=v[:, :], in0=u[:, :],
                                    scalar1=float(1 / 12), scalar2=0.5,
                                    op0=OP.mult, op1=OP.add)
            nc.vector.tensor_tensor(out=v[:, :], in0=v[:, :], in1=u[:, :],
                                    op=OP.mult)
            nc.vector.tensor_tensor(out=v[:, :], in0=lnt[:, :], in1=v[:, :],
                                    op=OP.subtract)
            wg = sm.tile([P, NG], f32)
            nc.vector.tensor_scalar(out=wg[:, :], in0=v[:, :],
                                    scalar1=float(-1.0 / B), scalar2=C0,
                                    op0=OP.mult, op1=OP.add)
            prod = sm.tile([P, NG], f32)
            nc.vector.tensor_tensor(out=prod[:, :], in0=wg[:, :],
                                    in1=losses[:, :], op=OP.mult)

            prow = sm.tile([P, 1], f32)
            nc.vector.reduce_sum(prow[:, :], prod[:, :], axis=AX)
            pscal = ps.tile([1, 1], f32, tag="pscal")
            nc.tensor.matmul(pscal[:, :], lhsT=prow[:, :], rhs=ones_col[:, :],
                             start=True, stop=True)
            psb = sm.tile([1, 1], f32)
            nc.scalar.copy(psb[:, :], pscal[:, :])
            nc.sync.dma_start(out[:, :], psb[:, :])

            if debug:
                for nm, tl in [("dkeys", keys), ("dloss", losses),
                               ("dranks", ranks), ("dwg", wg), ("dxt", xt)]:
                    dt_ = nc.dram_tensor(nm, list(tl.shape), f32,
                                         kind="ExternalOutput")
                    nc.sync.dma_start(dt_[:, :], tl[:, :])

    nc.finalize()
    return nc


def _shard_inputs(input: np.ndarray, target: np.ndarray):
    xin = np.ascontiguousarray(input, dtype=np.float32)
    toff = (np.arange(B, dtype=np.int64) % BL) * C + target.astype(np.int64)
    toff = toff.astype(np.int32).reshape(NCORES, NG, P)
    # tile[p, g] = row g*P+p  ->  flat host order (p, g)
    toff = np.ascontiguousarray(toff.transpose(0, 2, 1)).reshape(NCORES, BL)
    return [
        {"x": xin[c * BL:(c + 1) * BL], "toff": toff[c]}
        for c in range(NCORES)
    ]


def _run(input: np.ndarray, target: np.ndarray, trace: bool = False):
    from concourse.bass_utils import run_bass_kernel_spmd

    if "nc" not in _CACHE:
        _CACHE["nc"] = _build()
    nc = _CACHE["nc"]

    in_maps = _shard_inputs(input, target)
    res = run_bass_kernel_spmd(nc, in_maps, core_ids=list(range(NCORES)),
                               trace=trace)
    parts = [r["out"][0, 0] for r in res.results]
    total = np.float32(np.sum(np.asarray(parts, dtype=np.float64)))
    return np.asarray(total, dtype=np.float32), res


def kernel(input: np.ndarray, target: np.ndarray) -> np.ndarray:
    out, _ = _run(input, target, trace=False)
    return out

